# revision 1
# baseline (speedup 1.0000x reference)
"""SGC (2-hop simple graph convolution) Trainium2 kernel, 8-core SPMD.

out = S S x W^T + b,  S = D^{-1/2} (A + I) D^{-1/2}   (D = in-degree + 1)

Strategy:
  * project first: y = x @ W^T (64 ch), exact by associativity
  * factor norms:  S z = dinv * (A+I) (dinv * z)  -> per-node scalings only,
    messages are unweighted; self loop handled as a local add
  * per core: own 1/8 of destination nodes; edges partitioned by dst
  * gather sources with gpsimd dma_gather from an AllGather'ed bf16 table
    (rows padded to 128 ch = 256 B to satisfy elem%256 and int16 idx needs
    the table split in two 32768-row halves -> two message streams A/B)
  * scatter-adds via PE matmul: 128-message tiles x host-built 0/1 one-hot
    stationary tiles (shipped fp8, DMA-cast to bf16), accumulated in PSUM
    per 128-destination window; out-of-window slots give all-zero rows so
    stream tiles may straddle windows with no padding
  * node numbering inside tables is permuted (n -> p*WIN+w) so SBUF staging
    [128p, WIN, ch] maps contiguously to DRAM; host un-permutes at the end
"""

import sys

sys.path.insert(0, "/opt/trn_rl_repo")

import numpy as np
import ml_dtypes

# ---------------- problem constants (overridden by tests for small runs) ----
CFG = dict(
    N_NODES=65536,
    N_EDGES=655360,
    IN_C=128,
    OUT_C=64,
    CORES=8,
    CH=32,  # gather tiles (128 msgs each) per dma_gather call
    CHP=32,  # one-hot pairs per DMA chunk
    OC_PAD=128,  # bf16 channels per gather-table row (256 B)
    MSG_BUFS=3,
    OH_BUFS=3,
    PSUM_BUFS=6,
    RESYNC_G=16,
    ACT_EVAC=1,
    STAGE=6,
    SP=True,  # single_packet on gathers (safe only for num_idxs <= 1024)  # debug: 1 proj, 2 +ag1, 3 +gather/oh, 4 +hop1 mm, 5 +ag2, 6 full
)

SENT = 1 << 20  # sentinel "dst" for pad rows -> all-zero one-hot everywhere

FP8_ONE = 0x38  # float8_e4m3 bit pattern of 1.0


class Prep:
    pass


def _row_of_node(n, NP, WIN):
    # node n -> permuted gather-table row: shard base + p*WIN + w
    r = n % NP
    p = r % 128
    w = r // 128
    return (n // NP) * NP + p * WIN + w


def _preprocess(edge_index):
    N = CFG["N_NODES"]
    C = CFG["CORES"]
    NP = N // C
    WIN = NP // 128
    HALF = N // 2

    src = np.asarray(edge_index[0], dtype=np.int64)
    dst = np.asarray(edge_index[1], dtype=np.int64)
    deg = np.bincount(dst, minlength=N).astype(np.float32) + 1.0

    row_of = _row_of_node(np.arange(N, dtype=np.int64), NP, WIN)

    pr = Prep()
    pr.N, pr.C, pr.NP, pr.WIN, pr.HALF = N, C, NP, WIN, HALF

    # per-core, per-stream sorted message lists
    core_ld = [[None, None] for _ in range(C)]  # local dst per stream
    core_idx = [[None, None] for _ in range(C)]  # table idx per stream
    for i in range(C):
        m = (dst >= i * NP) & (dst < (i + 1) * NP)
        s_i = src[m]
        ld_i = dst[m] - i * NP
        order = np.argsort(ld_i, kind="stable")
        s_i, ld_i = s_i[order], ld_i[order]
        rows = row_of[s_i]
        a = rows < HALF
        core_ld[i][0], core_idx[i][0] = ld_i[a], rows[a]
        core_ld[i][1], core_idx[i][1] = ld_i[~a], rows[~a] - HALF

    # re-align all cores' streams at every RESYNC_G windows: within a group,
    # pad each core's segment to the max core's tile count, so tile t sits in
    # the same window neighborhood on every core (cuts union-pair straddle).
    G = CFG.get("RESYNC_G", 16)
    n_groups = (WIN + G - 1) // G
    for s in range(2):
        seg_tiles = np.zeros(n_groups, dtype=np.int64)
        for g in range(n_groups):
            lo, hi = g * G * 128, min((g + 1) * G, WIN) * 128
            for i in range(C):
                cnt = int(((core_ld[i][s] >= lo) & (core_ld[i][s] < hi)).sum())
                seg_tiles[g] = max(seg_tiles[g], (cnt + 127) // 128)
        for i in range(C):
            lds, ixs = [], []
            for g in range(n_groups):
                lo, hi = g * G * 128, min((g + 1) * G, WIN) * 128
                m = (core_ld[i][s] >= lo) & (core_ld[i][s] < hi)
                ld_g, ix_g = core_ld[i][s][m], core_idx[i][s][m]
                pad = int(seg_tiles[g]) * 128 - len(ld_g)
                lds.append(np.concatenate([ld_g, np.full(pad, SENT, np.int64)]))
                ixs.append(np.concatenate([ix_g, np.zeros(pad, np.int64)]))
            core_ld[i][s] = np.concatenate(lds)
            core_idx[i][s] = np.concatenate(ixs)
    T = [len(core_ld[0][0]) // 128, len(core_ld[0][1]) // 128]
    pr.T = T

    for i in range(C):
        for s in range(2):
            assert len(core_ld[i][s]) == T[s] * 128

    # union pair structure (w, stream, tile) across cores
    pair_set = set()
    for i in range(C):
        for s in range(2):
            L = core_ld[i][s].reshape(T[s], 128)
            for t in range(T[s]):
                real = L[t][L[t] != SENT]
                if len(real) == 0:
                    continue
                for w in range(int(real.min()) // 128, int(real.max()) // 128 + 1):
                    pair_set.add((w, s, t))
    for w in range(WIN):  # every window needs >=1 pair so psum gets reset
        if not any(p[0] == w for p in pair_set):
            pair_set.add((w, 0, 0))
    pairs = sorted(pair_set)
    pr.pairs = pairs
    pr.n_pairs = len(pairs)
    segs = [[] for _ in range(WIN)]
    for k, (w, s, t) in enumerate(pairs):
        segs[w].append(k)
    pr.segs = segs

    # per-core one-hot tiles [128, n_pairs, 128] fp8(0/1)
    pr.onehot = []
    pr.idx_wrapped = []
    pr.deg_staged = []
    for i in range(C):
        oh = np.zeros((128, pr.n_pairs, 128), dtype=np.uint8)
        for k, (w, s, t) in enumerate(pairs):
            ld_t = core_ld[i][s][t * 128 : (t + 1) * 128]
            slot = ld_t - 128 * w
            valid = (slot >= 0) & (slot < 128)
            rr = np.nonzero(valid)[0]
            oh[rr, k, slot[rr]] = FP8_ONE
        pr.onehot.append(oh.view(ml_dtypes.float8_e4m3fn))

        blocks = []
        for s in range(2):
            ix = core_idx[i][s].astype(np.int16)
            assert (core_idx[i][s] < 32768).all() and (core_idx[i][s] >= 0).all()
            w16 = ix.reshape(-1, 16).T  # [16, T*8]
            blocks.append(np.tile(w16, (8, 1)))  # replicate to 128 partitions
        pr.idx_wrapped.append(
            np.ascontiguousarray(np.concatenate(blocks, axis=1))
        )

        dshard = deg[i * NP : (i + 1) * NP]
        pr.deg_staged.append(
            np.ascontiguousarray(dshard.reshape(WIN, 128).T.astype(np.float32))
        )

    return pr


# ------------------------------------------------------------------ bass ----


def _build(pr):
    import concourse.bass as bass
    import concourse.bacc as bacc
    import concourse.mybir as mybir
    import concourse.tile as tile
    from concourse._compat import get_trn_type

    dt = mybir.dt
    Alu = mybir.AluOpType
    F32, BF16, FP8, I16 = dt.float32, dt.bfloat16, dt.float8e4, dt.int16

    IN_C, OUT_C = CFG["IN_C"], CFG["OUT_C"]
    OC_PAD, CH, CHP = CFG["OC_PAD"], CFG["CH"], CFG["CHP"]
    N, C, NP, WIN, HALF = pr.N, pr.C, pr.NP, pr.WIN, pr.HALF
    T = pr.T

    nc = bacc.Bacc(
        get_trn_type() or "TRN2",
        target_bir_lowering=False,
        debug=False,
        num_devices=C,
    )

    xt_d = nc.dram_tensor("xt", [IN_C, NP], F32, kind="ExternalInput")
    wt_d = nc.dram_tensor("wt", [IN_C, OUT_C], F32, kind="ExternalInput")
    b_d = nc.dram_tensor("bias", [128, OUT_C], F32, kind="ExternalInput")
    deg_d = nc.dram_tensor("deg", [128, WIN], F32, kind="ExternalInput")
    idx_d = nc.dram_tensor(
        "idx", [128, (T[0] + T[1]) * 8], I16, kind="ExternalInput"
    )
    oh_d = nc.dram_tensor("oh", [128, pr.n_pairs, 128], FP8, kind="ExternalInput")
    out_d = nc.dram_tensor("out", [NP, OUT_C], F32, kind="ExternalOutput")

    rg = [list(range(C))]

    with tile.TileContext(nc) as tc:
        with (
            tc.tile_pool(name="const", bufs=1) as const,
            tc.tile_pool(name="dram", bufs=1, space="DRAM") as dram,
            tc.tile_pool(name="psum_y", bufs=2, space="PSUM") as psum_y,
            tc.tile_pool(name="psum_w", bufs=CFG["PSUM_BUFS"], space="PSUM") as psum_w,
            tc.tile_pool(name="msgA", bufs=CFG["MSG_BUFS"]) as msgA_pool,
            tc.tile_pool(name="msgB", bufs=CFG["MSG_BUFS"]) as msgB_pool,
            tc.tile_pool(name="ohp", bufs=CFG["OH_BUFS"]) as oh_pool,
            tc.tile_pool(name="tmp", bufs=4) as tmp_pool,
        ):
            cc1_in = dram.tile([NP, OC_PAD], BF16)
            cc1_out = dram.tile([N, OC_PAD], BF16, addr_space="Shared")
            cc2_in = dram.tile([NP, OC_PAD], BF16)
            cc2_out = dram.tile([N, OC_PAD], BF16, addr_space="Shared")

            idx_sb = const.tile([128, (T[0] + T[1]) * 8], I16)
            nc.sync.dma_start(idx_sb[:], idx_d[:])
            wt_sb = const.tile([IN_C, OUT_C], F32)
            nc.sync.dma_start(wt_sb[:], wt_d[:])
            b_sb = const.tile([128, OUT_C], F32)
            nc.sync.dma_start(b_sb[:], b_d[:])
            deg_sb = const.tile([128, WIN], F32)
            nc.sync.dma_start(deg_sb[:], deg_d[:])
            xt_sb = const.tile([IN_C, NP], F32)
            nc.sync.dma_start(xt_sb[:], xt_d[:])

            deginv = const.tile([128, WIN], F32)
            nc.vector.reciprocal(deginv[:], deg_sb[:])
            dinv = const.tile([128, WIN], F32)
            nc.scalar.activation(
                dinv[:], deginv[:], mybir.ActivationFunctionType.Sqrt
            )

            z0f = const.tile([128, WIN, OUT_C], F32)
            z1f = const.tile([128, WIN, OUT_C], F32)
            outst = const.tile([128, WIN, OUT_C], F32)
            zpad1 = const.tile([128, WIN, OC_PAD], BF16)
            zpad2 = const.tile([128, WIN, OC_PAD], BF16)
            nc.vector.memset(zpad1[:], 0.0)
            nc.vector.memset(zpad2[:], 0.0)

            # ---- projection: z0 = dinv * (x @ W^T), staged [p, w, ch] ----
            for r in range(WIN):
                py = psum_y.tile([128, OUT_C], F32)
                nc.tensor.matmul(
                    py[:],
                    xt_sb[:, r * 128 : (r + 1) * 128],
                    wt_sb[:],
                    start=True,
                    stop=True,
                )
                if CFG["ACT_EVAC"]:
                    nc.scalar.mul(z0f[:, r, :], py[:], dinv[:, r : r + 1])
                    nc.scalar.copy(zpad1[:, r, 0:OUT_C], z0f[:, r, :])
                else:
                    nc.vector.tensor_scalar(
                        z0f[:, r, :], py[:], dinv[:, r : r + 1], None, Alu.mult
                    )
                    nc.vector.tensor_copy(zpad1[:, r, 0:OUT_C], z0f[:, r, :])
            STAGE = CFG["STAGE"]
            if STAGE >= 2:
                nc.sync.dma_start(cc1_in[:], zpad1[:])
                nc.gpsimd.collective_compute(
                    "AllGather",
                    Alu.bypass,
                    replica_groups=rg,
                    ins=[cc1_in.opt()],
                    outs=[cc1_out.opt()],
                )

            calls = [(T[0] + CH - 1) // CH, (T[1] + CH - 1) // CH]
            n_oh_chunks = (pr.n_pairs + CHP - 1) // CHP
            colbase = [0, T[0] * 8]

            def run_hop(cc_out, evac, do_mm=True):
                tabs = [cc_out[0:HALF, :], cc_out[HALF : 2 * HALF, :]]
                pools = [msgA_pool, msgB_pool]
                msg_tiles = [{}, {}]
                oh_tiles = {}
                next_call = [0, 0]
                next_oh = [0]

                def emit_gather(s):
                    c = next_call[s]
                    ntiles = min(CH, T[s] - c * CH)
                    ni = ntiles * 128
                    t = pools[s].tile([128, CH, OC_PAD], BF16, tag=f"msg{s}")
                    sl = slice(colbase[s] + c * CH * 8, colbase[s] + c * CH * 8 + ntiles * 8)
                    nc.gpsimd.dma_gather(
                        t[:, 0:ntiles, :],
                        tabs[s],
                        idx_sb[:, sl],
                        ni,
                        ni,
                        OC_PAD,
                        single_packet=(ni <= 1024),
                    )
                    msg_tiles[s][c] = t
                    next_call[s] = c + 1

                def emit_oh():
                    k = next_oh[0]
                    npair = min(CHP, pr.n_pairs - k * CHP)
                    t = oh_pool.tile([128, CHP, 128], BF16, tag="oh")
                    nc.gpsimd.dma_start(
                        out=t[:, 0:npair, :],
                        in_=oh_d[:, k * CHP : k * CHP + npair, :],
                    )
                    oh_tiles[k] = t
                    next_oh[0] = k + 1

                for w in range(WIN):
                    seg = pr.segs[w]
                    # make sure resources (plus one chunk of prefetch) exist
                    for pk in seg:
                        _, s, t = pr.pairs[pk]
                        while next_call[s] <= min(t // CH + 1, calls[s] - 1):
                            emit_gather(s)
                        while next_oh[0] <= min(pk // CHP + 1, n_oh_chunks - 1):
                            emit_oh()
                    if not do_mm:
                        continue
                    pw = psum_w.tile([128, OUT_C], F32)
                    for j, pk in enumerate(seg):
                        _, s, t = pr.pairs[pk]
                        oh_ap = oh_tiles[pk // CHP][:, pk % CHP, :]
                        msg_ap = msg_tiles[s][t // CH][:, t % CH, 0:OUT_C]
                        nc.tensor.matmul(
                            pw[:],
                            oh_ap,
                            msg_ap,
                            start=(j == 0),
                            stop=(j == len(seg) - 1),
                        )
                    evac(w, pw)

            # ---- hop 1:  z1 = (psum + z0) / deg ----
            def evac1(w, pw):
                tmp = tmp_pool.tile([128, OUT_C], F32, tag="tmp")
                nc.vector.tensor_add(tmp[:], pw[:], z0f[:, w, :])
                if CFG["ACT_EVAC"]:
                    nc.scalar.mul(z1f[:, w, :], tmp[:], deginv[:, w : w + 1])
                    nc.scalar.mul(
                        zpad2[:, w, 0:OUT_C], tmp[:], deginv[:, w : w + 1]
                    )
                else:
                    nc.vector.tensor_scalar(
                        z1f[:, w, :], tmp[:], deginv[:, w : w + 1], None, Alu.mult
                    )
                    nc.vector.tensor_copy(zpad2[:, w, 0:OUT_C], z1f[:, w, :])

            if STAGE >= 3:
                run_hop(cc1_out, evac1, do_mm=STAGE >= 4)
            if STAGE >= 5:
                nc.sync.dma_start(cc2_in[:], zpad2[:])
                nc.gpsimd.collective_compute(
                    "AllGather",
                    Alu.bypass,
                    replica_groups=rg,
                    ins=[cc2_in.opt()],
                    outs=[cc2_out.opt()],
                )

            # ---- hop 2:  out = dinv * (psum + z1) + b ----
            def evac2(w, pw):
                tmp = tmp_pool.tile([128, OUT_C], F32, tag="tmp")
                tmp2 = tmp_pool.tile([128, OUT_C], F32, tag="tmp2")
                nc.vector.tensor_add(tmp[:], pw[:], z1f[:, w, :])
                if CFG["ACT_EVAC"]:
                    nc.scalar.mul(tmp2[:], tmp[:], dinv[:, w : w + 1])
                else:
                    nc.vector.tensor_scalar(
                        tmp2[:], tmp[:], dinv[:, w : w + 1], None, Alu.mult
                    )
                nc.vector.tensor_add(outst[:, w, :], tmp2[:], b_sb[:])

            if STAGE >= 6:
                run_hop(cc2_out, evac2)
            src_final = {1: z0f, 2: z0f, 3: z0f, 4: z1f, 5: z1f, 6: outst}[STAGE]
            nc.sync.dma_start(out_d[:], src_final[:])

    nc.compile()
    return nc


def _make_in_maps(pr, x, W, b):
    C, NP, WIN = pr.C, pr.NP, pr.WIN
    x = np.asarray(x, dtype=np.float32)
    W = np.asarray(W, dtype=np.float32)
    b = np.asarray(b, dtype=np.float32)
    wt = np.ascontiguousarray(W.T)
    b_rep = np.ascontiguousarray(np.broadcast_to(b, (128, len(b))))
    in_maps = []
    for i in range(C):
        xt = np.ascontiguousarray(x[i * NP : (i + 1) * NP].T)
        in_maps.append(
            dict(
                xt=xt,
                wt=wt,
                bias=b_rep,
                deg=pr.deg_staged[i],
                idx=pr.idx_wrapped[i],
                oh=pr.onehot[i],
            )
        )
    return in_maps


def _unpermute(o, pr):
    # device rows are p*WIN+w; node order is w*128+p
    return (
        o.reshape(128, pr.WIN, o.shape[-1])
        .transpose(1, 0, 2)
        .reshape(pr.NP, o.shape[-1])
    )


_CACHE = {}


def kernel(x, edge_index, W, b):
    pr = _preprocess(edge_index)
    nc = _build(pr)
    in_maps = _make_in_maps(pr, x, W, b)

    from concourse import bass_utils

    res = bass_utils.run_bass_kernel_spmd(
        nc, in_maps, core_ids=list(range(pr.C))
    )
    shards = [_unpermute(res.results[i]["out"], pr) for i in range(pr.C)]
    return np.ascontiguousarray(np.concatenate(shards, axis=0))



# revision 6
# speedup vs baseline: 1.8866x; 1.8866x over previous
"""SGC (2-hop simple graph convolution) Trainium2 kernel, 8-core SPMD.

out = S S x W^T + b,  S = D^{-1/2} (A + I) D^{-1/2}   (D = in-degree + 1)

Strategy:
  * project first: y = x @ W^T (64 ch), exact by associativity
  * factor norms:  S z = dinv * (A+I) (dinv * z)  -> per-node scalings only,
    messages are unweighted; self loop handled as a local add
  * per core: own 1/8 of destination nodes; edges partitioned by dst
  * gather sources with gpsimd dma_gather from an AllGather'ed bf16 table
    (rows padded to 128 ch = 256 B to satisfy elem%256 and int16 idx needs
    the table split in two 32768-row halves -> two message streams A/B)
  * scatter-adds via PE matmul: 128-message tiles x host-built 0/1 one-hot
    stationary tiles (shipped fp8, DMA-cast to bf16), accumulated in PSUM
    per 128-destination window; out-of-window slots give all-zero rows so
    stream tiles may straddle windows with no padding
  * node numbering inside tables is permuted (n -> p*WIN+w) so SBUF staging
    [128p, WIN, ch] maps contiguously to DRAM; host un-permutes at the end
"""

import sys

sys.path.insert(0, "/opt/trn_rl_repo")

import numpy as np
import ml_dtypes

# ---------------- problem constants (overridden by tests for small runs) ----
CFG = dict(
    N_NODES=65536,
    N_EDGES=655360,
    IN_C=128,
    OUT_C=64,
    CORES=8,
    CH=32,  # gather tiles (128 msgs each) per dma_gather call
    CHP=32,  # one-hot pairs per DMA chunk
    OC_PAD=128,  # bf16 channels per gather-table row (256 B)
    MSG_BUFS=3,
    OH_BUFS=3,
    PSUM_BUFS=6,
    RESYNC_G=16,
    ACT_EVAC=1,
    STAGE=6,
    SP=True,  # single_packet on gathers (safe only for num_idxs <= 1024)  # debug: 1 proj, 2 +ag1, 3 +gather/oh, 4 +hop1 mm, 5 +ag2, 6 full
    NQ=4,  # SWDGE queues for gathers (each queue = its own gpsimd cpu pair)
    OH_SYNC=1,  # one-hot shipped bf16, loaded on sync HWDGE (off gpsimd)
)

SENT = 1 << 20  # sentinel "dst" for pad rows -> all-zero one-hot everywhere

FP8_ONE = 0x38  # float8_e4m3 bit pattern of 1.0


class Prep:
    pass


def _row_of_node(n, NP, WIN):
    # node n -> permuted gather-table row: shard base + p*WIN + w
    r = n % NP
    p = r % 128
    w = r // 128
    return (n // NP) * NP + p * WIN + w


def _preprocess(edge_index):
    N = CFG["N_NODES"]
    C = CFG["CORES"]
    NP = N // C
    WIN = NP // 128
    HALF = N // 2

    src = np.asarray(edge_index[0], dtype=np.int64)
    dst = np.asarray(edge_index[1], dtype=np.int64)
    deg = np.bincount(dst, minlength=N).astype(np.float32) + 1.0

    row_of = _row_of_node(np.arange(N, dtype=np.int64), NP, WIN)

    pr = Prep()
    pr.N, pr.C, pr.NP, pr.WIN, pr.HALF = N, C, NP, WIN, HALF

    # per-core, per-stream sorted message lists
    core_ld = [[None, None] for _ in range(C)]  # local dst per stream
    core_idx = [[None, None] for _ in range(C)]  # table idx per stream
    for i in range(C):
        m = (dst >= i * NP) & (dst < (i + 1) * NP)
        s_i = src[m]
        ld_i = dst[m] - i * NP
        order = np.argsort(ld_i, kind="stable")
        s_i, ld_i = s_i[order], ld_i[order]
        rows = row_of[s_i]
        a = rows < HALF
        core_ld[i][0], core_idx[i][0] = ld_i[a], rows[a]
        core_ld[i][1], core_idx[i][1] = ld_i[~a], rows[~a] - HALF

    # re-align all cores' streams at every RESYNC_G windows: within a group,
    # pad each core's segment to the max core's tile count, so tile t sits in
    # the same window neighborhood on every core (cuts union-pair straddle).
    G = CFG.get("RESYNC_G", 16)
    n_groups = (WIN + G - 1) // G
    for s in range(2):
        seg_tiles = np.zeros(n_groups, dtype=np.int64)
        for g in range(n_groups):
            lo, hi = g * G * 128, min((g + 1) * G, WIN) * 128
            for i in range(C):
                cnt = int(((core_ld[i][s] >= lo) & (core_ld[i][s] < hi)).sum())
                seg_tiles[g] = max(seg_tiles[g], (cnt + 127) // 128)
        for i in range(C):
            lds, ixs = [], []
            for g in range(n_groups):
                lo, hi = g * G * 128, min((g + 1) * G, WIN) * 128
                m = (core_ld[i][s] >= lo) & (core_ld[i][s] < hi)
                ld_g, ix_g = core_ld[i][s][m], core_idx[i][s][m]
                pad = int(seg_tiles[g]) * 128 - len(ld_g)
                lds.append(np.concatenate([ld_g, np.full(pad, SENT, np.int64)]))
                ixs.append(np.concatenate([ix_g, np.zeros(pad, np.int64)]))
            core_ld[i][s] = np.concatenate(lds)
            core_idx[i][s] = np.concatenate(ixs)
    T = [len(core_ld[0][0]) // 128, len(core_ld[0][1]) // 128]
    pr.T = T

    for i in range(C):
        for s in range(2):
            assert len(core_ld[i][s]) == T[s] * 128

    # union pair structure (w, stream, tile) across cores
    pair_set = set()
    for i in range(C):
        for s in range(2):
            L = core_ld[i][s].reshape(T[s], 128)
            for t in range(T[s]):
                real = L[t][L[t] != SENT]
                if len(real) == 0:
                    continue
                for w in range(int(real.min()) // 128, int(real.max()) // 128 + 1):
                    pair_set.add((w, s, t))
    for w in range(WIN):  # every window needs >=1 pair so psum gets reset
        if not any(p[0] == w for p in pair_set):
            pair_set.add((w, 0, 0))
    pairs = sorted(pair_set)
    pr.pairs = pairs
    pr.n_pairs = len(pairs)
    segs = [[] for _ in range(WIN)]
    for k, (w, s, t) in enumerate(pairs):
        segs[w].append(k)
    pr.segs = segs

    # per-core one-hot tiles [128, n_pairs, 128] fp8(0/1)
    pr.onehot = []
    pr.idx_wrapped = []
    pr.deg_staged = []
    oh_sync = CFG.get("OH_SYNC", 0)
    BF16_ONE = 0x3F80  # bfloat16 bit pattern of 1.0
    one_bits = BF16_ONE if oh_sync else FP8_ONE
    oh_dt = np.uint16 if oh_sync else np.uint8
    oh_view = ml_dtypes.bfloat16 if oh_sync else ml_dtypes.float8_e4m3fn
    for i in range(C):
        oh = np.zeros((128, pr.n_pairs, 128), dtype=oh_dt)
        for k, (w, s, t) in enumerate(pairs):
            ld_t = core_ld[i][s][t * 128 : (t + 1) * 128]
            slot = ld_t - 128 * w
            valid = (slot >= 0) & (slot < 128)
            rr = np.nonzero(valid)[0]
            oh[rr, k, slot[rr]] = one_bits
        pr.onehot.append(oh.view(oh_view))

        blocks = []
        for s in range(2):
            ix = core_idx[i][s].astype(np.int16)
            assert (core_idx[i][s] < 32768).all() and (core_idx[i][s] >= 0).all()
            w16 = ix.reshape(-1, 16).T  # [16, T*8]
            blocks.append(np.tile(w16, (8, 1)))  # replicate to 128 partitions
        pr.idx_wrapped.append(
            np.ascontiguousarray(np.concatenate(blocks, axis=1))
        )

        dshard = deg[i * NP : (i + 1) * NP]
        pr.deg_staged.append(
            np.ascontiguousarray(dshard.reshape(WIN, 128).T.astype(np.float32))
        )

    return pr


# ------------------------------------------------------------------ bass ----


def _build(pr):
    import concourse.bass as bass
    import concourse.bacc as bacc
    import concourse.mybir as mybir
    import concourse.tile as tile
    from concourse._compat import get_trn_type

    dt = mybir.dt
    Alu = mybir.AluOpType
    F32, BF16, FP8, I16 = dt.float32, dt.bfloat16, dt.float8e4, dt.int16

    IN_C, OUT_C = CFG["IN_C"], CFG["OUT_C"]
    OC_PAD, CH, CHP = CFG["OC_PAD"], CFG["CH"], CFG["CHP"]
    N, C, NP, WIN, HALF = pr.N, pr.C, pr.NP, pr.WIN, pr.HALF
    T = pr.T

    nc = bacc.Bacc(
        get_trn_type() or "TRN2",
        target_bir_lowering=False,
        debug=False,
        num_devices=C,
        num_swdge_queues=CFG.get("NQ", 1),
    )

    xt_d = nc.dram_tensor("xt", [IN_C, NP], F32, kind="ExternalInput")
    wt_d = nc.dram_tensor("wt", [IN_C, OUT_C], F32, kind="ExternalInput")
    b_d = nc.dram_tensor("bias", [128, OUT_C], F32, kind="ExternalInput")
    deg_d = nc.dram_tensor("deg", [128, WIN], F32, kind="ExternalInput")
    idx_d = nc.dram_tensor(
        "idx", [128, (T[0] + T[1]) * 8], I16, kind="ExternalInput"
    )
    OH_DT = BF16 if CFG.get("OH_SYNC", 0) else FP8
    oh_d = nc.dram_tensor("oh", [128, pr.n_pairs, 128], OH_DT, kind="ExternalInput")
    out_d = nc.dram_tensor("out", [NP, OUT_C], F32, kind="ExternalOutput")

    rg = [list(range(C))]

    with tile.TileContext(nc) as tc:
        with (
            tc.tile_pool(name="const", bufs=1) as const,
            tc.tile_pool(name="dram", bufs=1, space="DRAM") as dram,
            tc.tile_pool(name="psum_y", bufs=2, space="PSUM") as psum_y,
            tc.tile_pool(name="psum_w", bufs=CFG["PSUM_BUFS"], space="PSUM") as psum_w,
            tc.tile_pool(name="msgA", bufs=CFG["MSG_BUFS"]) as msgA_pool,
            tc.tile_pool(name="msgB", bufs=CFG["MSG_BUFS"]) as msgB_pool,
            tc.tile_pool(name="ohp", bufs=CFG["OH_BUFS"]) as oh_pool,
            tc.tile_pool(name="tmp", bufs=4) as tmp_pool,
        ):
            cc1_in = dram.tile([NP, OC_PAD], BF16)
            cc1_out = dram.tile([N, OC_PAD], BF16, addr_space="Shared")
            cc2_in = dram.tile([NP, OC_PAD], BF16)
            cc2_out = dram.tile([N, OC_PAD], BF16, addr_space="Shared")

            idx_sb = const.tile([128, (T[0] + T[1]) * 8], I16)
            nc.sync.dma_start(idx_sb[:], idx_d[:])
            wt_sb = const.tile([IN_C, OUT_C], F32)
            nc.sync.dma_start(wt_sb[:], wt_d[:])
            b_sb = const.tile([128, OUT_C], F32)
            nc.sync.dma_start(b_sb[:], b_d[:])
            deg_sb = const.tile([128, WIN], F32)
            nc.sync.dma_start(deg_sb[:], deg_d[:])
            xt_sb = const.tile([IN_C, NP], F32)
            nc.sync.dma_start(xt_sb[:], xt_d[:])

            deginv = const.tile([128, WIN], F32)
            nc.vector.reciprocal(deginv[:], deg_sb[:])
            dinv = const.tile([128, WIN], F32)
            nc.scalar.activation(
                dinv[:], deginv[:], mybir.ActivationFunctionType.Sqrt
            )

            z0f = const.tile([128, WIN, OUT_C], F32)
            z1f = const.tile([128, WIN, OUT_C], F32)
            outst = const.tile([128, WIN, OUT_C], F32)
            zpad1 = const.tile([128, WIN, OC_PAD], BF16)
            zpad2 = const.tile([128, WIN, OC_PAD], BF16)
            nc.vector.memset(zpad1[:], 0.0)
            nc.vector.memset(zpad2[:], 0.0)

            # ---- projection: z0 = dinv * (x @ W^T), staged [p, w, ch] ----
            for r in range(WIN):
                py = psum_y.tile([128, OUT_C], F32)
                nc.tensor.matmul(
                    py[:],
                    xt_sb[:, r * 128 : (r + 1) * 128],
                    wt_sb[:],
                    start=True,
                    stop=True,
                )
                if CFG["ACT_EVAC"]:
                    nc.scalar.mul(z0f[:, r, :], py[:], dinv[:, r : r + 1])
                    nc.scalar.copy(zpad1[:, r, 0:OUT_C], z0f[:, r, :])
                else:
                    nc.vector.tensor_scalar(
                        z0f[:, r, :], py[:], dinv[:, r : r + 1], None, Alu.mult
                    )
                    nc.vector.tensor_copy(zpad1[:, r, 0:OUT_C], z0f[:, r, :])
            STAGE = CFG["STAGE"]
            if STAGE >= 2:
                nc.sync.dma_start(cc1_in[:], zpad1[:])
                nc.gpsimd.collective_compute(
                    "AllGather",
                    Alu.bypass,
                    replica_groups=rg,
                    ins=[cc1_in.opt()],
                    outs=[cc1_out.opt()],
                )

            calls = [(T[0] + CH - 1) // CH, (T[1] + CH - 1) // CH]
            n_oh_chunks = (pr.n_pairs + CHP - 1) // CHP
            colbase = [0, T[0] * 8]

            NQ = CFG.get("NQ", 1)
            gq_counter = [0]

            def run_hop(cc_out, evac, do_mm=True):
                tabs = [cc_out[0:HALF, :], cc_out[HALF : 2 * HALF, :]]
                pools = [msgA_pool, msgB_pool]
                msg_tiles = [{}, {}]
                oh_tiles = {}
                next_call = [0, 0]
                next_oh = [0]

                def emit_gather(s):
                    c = next_call[s]
                    ntiles = min(CH, T[s] - c * CH)
                    ni = ntiles * 128
                    t = pools[s].tile([128, CH, OC_PAD], BF16, tag=f"msg{s}")
                    sl = slice(colbase[s] + c * CH * 8, colbase[s] + c * CH * 8 + ntiles * 8)
                    nc.gpsimd.dma_gather(
                        t[:, 0:ntiles, :],
                        tabs[s],
                        idx_sb[:, sl],
                        ni,
                        ni,
                        OC_PAD,
                        single_packet=(ni <= 1024),
                        queue_num=gq_counter[0] % NQ,
                    )
                    gq_counter[0] += 1
                    msg_tiles[s][c] = t
                    next_call[s] = c + 1

                def emit_oh():
                    k = next_oh[0]
                    npair = min(CHP, pr.n_pairs - k * CHP)
                    t = oh_pool.tile([128, CHP, 128], BF16, tag="oh")
                    oh_eng = nc.sync if CFG.get("OH_SYNC", 0) else nc.gpsimd
                    oh_eng.dma_start(
                        out=t[:, 0:npair, :],
                        in_=oh_d[:, k * CHP : k * CHP + npair, :],
                    )
                    oh_tiles[k] = t
                    next_oh[0] = k + 1

                for w in range(WIN):
                    seg = pr.segs[w]
                    # make sure resources (plus one chunk of prefetch) exist
                    for pk in seg:
                        _, s, t = pr.pairs[pk]
                        while next_call[s] <= min(t // CH + 1, calls[s] - 1):
                            emit_gather(s)
                        while next_oh[0] <= min(pk // CHP + 1, n_oh_chunks - 1):
                            emit_oh()
                    if not do_mm:
                        continue
                    pw = psum_w.tile([128, OUT_C], F32)
                    for j, pk in enumerate(seg):
                        _, s, t = pr.pairs[pk]
                        oh_ap = oh_tiles[pk // CHP][:, pk % CHP, :]
                        msg_ap = msg_tiles[s][t // CH][:, t % CH, 0:OUT_C]
                        nc.tensor.matmul(
                            pw[:],
                            oh_ap,
                            msg_ap,
                            start=(j == 0),
                            stop=(j == len(seg) - 1),
                        )
                    evac(w, pw)

            # ---- hop 1:  z1 = (psum + z0) / deg ----
            def evac1(w, pw):
                tmp = tmp_pool.tile([128, OUT_C], F32, tag="tmp")
                nc.vector.tensor_add(tmp[:], pw[:], z0f[:, w, :])
                if CFG["ACT_EVAC"]:
                    nc.scalar.mul(z1f[:, w, :], tmp[:], deginv[:, w : w + 1])
                    nc.scalar.mul(
                        zpad2[:, w, 0:OUT_C], tmp[:], deginv[:, w : w + 1]
                    )
                else:
                    nc.vector.tensor_scalar(
                        z1f[:, w, :], tmp[:], deginv[:, w : w + 1], None, Alu.mult
                    )
                    nc.vector.tensor_copy(zpad2[:, w, 0:OUT_C], z1f[:, w, :])

            if STAGE >= 3:
                run_hop(cc1_out, evac1, do_mm=STAGE >= 4)
            if STAGE >= 5:
                nc.sync.dma_start(cc2_in[:], zpad2[:])
                nc.gpsimd.collective_compute(
                    "AllGather",
                    Alu.bypass,
                    replica_groups=rg,
                    ins=[cc2_in.opt()],
                    outs=[cc2_out.opt()],
                )

            # ---- hop 2:  out = dinv * (psum + z1) + b ----
            def evac2(w, pw):
                tmp = tmp_pool.tile([128, OUT_C], F32, tag="tmp")
                tmp2 = tmp_pool.tile([128, OUT_C], F32, tag="tmp2")
                nc.vector.tensor_add(tmp[:], pw[:], z1f[:, w, :])
                if CFG["ACT_EVAC"]:
                    nc.scalar.mul(tmp2[:], tmp[:], dinv[:, w : w + 1])
                else:
                    nc.vector.tensor_scalar(
                        tmp2[:], tmp[:], dinv[:, w : w + 1], None, Alu.mult
                    )
                nc.vector.tensor_add(outst[:, w, :], tmp2[:], b_sb[:])

            if STAGE >= 6:
                run_hop(cc2_out, evac2)
            src_final = {1: z0f, 2: z0f, 3: z0f, 4: z1f, 5: z1f, 6: outst}[STAGE]
            nc.sync.dma_start(out_d[:], src_final[:])

    nc.compile()
    return nc


def _make_in_maps(pr, x, W, b):
    C, NP, WIN = pr.C, pr.NP, pr.WIN
    x = np.asarray(x, dtype=np.float32)
    W = np.asarray(W, dtype=np.float32)
    b = np.asarray(b, dtype=np.float32)
    wt = np.ascontiguousarray(W.T)
    b_rep = np.ascontiguousarray(np.broadcast_to(b, (128, len(b))))
    in_maps = []
    for i in range(C):
        xt = np.ascontiguousarray(x[i * NP : (i + 1) * NP].T)
        in_maps.append(
            dict(
                xt=xt,
                wt=wt,
                bias=b_rep,
                deg=pr.deg_staged[i],
                idx=pr.idx_wrapped[i],
                oh=pr.onehot[i],
            )
        )
    return in_maps


def _unpermute(o, pr):
    # device rows are p*WIN+w; node order is w*128+p
    return (
        o.reshape(128, pr.WIN, o.shape[-1])
        .transpose(1, 0, 2)
        .reshape(pr.NP, o.shape[-1])
    )


_CACHE = {}


def kernel(x, edge_index, W, b):
    pr = _preprocess(edge_index)
    nc = _build(pr)
    in_maps = _make_in_maps(pr, x, W, b)

    from concourse import bass_utils

    res = bass_utils.run_bass_kernel_spmd(
        nc, in_maps, core_ids=list(range(pr.C))
    )
    shards = [_unpermute(res.results[i]["out"], pr) for i in range(pr.C)]
    return np.ascontiguousarray(np.concatenate(shards, axis=0))



# revision 8
# speedup vs baseline: 2.0257x; 1.0737x over previous
"""SGC (2-hop simple graph convolution) Trainium2 kernel, 8-core SPMD.

out = S S x W^T + b,  S = D^{-1/2} (A + I) D^{-1/2}   (D = in-degree + 1)

Strategy:
  * project first: y = x @ W^T (64 ch), exact by associativity
  * factor norms:  S z = dinv * (A+I) (dinv * z)  -> per-node scalings only,
    messages are unweighted; self loop handled as a local add
  * per core: own 1/8 of destination nodes; edges partitioned by dst
  * gather sources with gpsimd dma_gather from an AllGather'ed bf16 table
    (rows padded to 128 ch = 256 B to satisfy elem%256 and int16 idx needs
    the table split in two 32768-row halves -> two message streams A/B)
  * scatter-adds via PE matmul: 128-message tiles x host-built 0/1 one-hot
    stationary tiles (shipped fp8, DMA-cast to bf16), accumulated in PSUM
    per 128-destination window; out-of-window slots give all-zero rows so
    stream tiles may straddle windows with no padding
  * node numbering inside tables is permuted (n -> p*WIN+w) so SBUF staging
    [128p, WIN, ch] maps contiguously to DRAM; host un-permutes at the end
"""

import sys

sys.path.insert(0, "/opt/trn_rl_repo")

import numpy as np
import ml_dtypes

# ---------------- problem constants (overridden by tests for small runs) ----
CFG = dict(
    N_NODES=65536,
    N_EDGES=655360,
    IN_C=128,
    OUT_C=64,
    CORES=8,
    CH=16,  # gather tiles (128 msgs each) per dma_gather call
    CHP=32,  # one-hot pairs per DMA chunk
    OC_PAD=128,  # bf16 channels per gather-table row (256 B)
    MSG_BUFS=6,
    OH_BUFS=3,
    PF=3,  # gather chunks prefetched ahead of demand

    PSUM_BUFS=6,
    RESYNC_G=16,
    ACT_EVAC=1,
    STAGE=6,
    SP=True,  # single_packet on gathers (safe only for num_idxs <= 1024)  # debug: 1 proj, 2 +ag1, 3 +gather/oh, 4 +hop1 mm, 5 +ag2, 6 full
    NQ=4,  # SWDGE queues for gathers (each queue = its own gpsimd cpu pair)
    OH_SYNC=1,  # one-hot shipped bf16, loaded on sync HWDGE (off gpsimd)
)

SENT = 1 << 20  # sentinel "dst" for pad rows -> all-zero one-hot everywhere

FP8_ONE = 0x38  # float8_e4m3 bit pattern of 1.0


class Prep:
    pass


def _row_of_node(n, NP, WIN):
    # node n -> permuted gather-table row: shard base + p*WIN + w
    r = n % NP
    p = r % 128
    w = r // 128
    return (n // NP) * NP + p * WIN + w


def _preprocess(edge_index):
    N = CFG["N_NODES"]
    C = CFG["CORES"]
    NP = N // C
    WIN = NP // 128
    HALF = N // 2

    src = np.asarray(edge_index[0], dtype=np.int64)
    dst = np.asarray(edge_index[1], dtype=np.int64)
    deg = np.bincount(dst, minlength=N).astype(np.float32) + 1.0

    row_of = _row_of_node(np.arange(N, dtype=np.int64), NP, WIN)

    pr = Prep()
    pr.N, pr.C, pr.NP, pr.WIN, pr.HALF = N, C, NP, WIN, HALF

    # per-core, per-stream sorted message lists
    core_ld = [[None, None] for _ in range(C)]  # local dst per stream
    core_idx = [[None, None] for _ in range(C)]  # table idx per stream
    for i in range(C):
        m = (dst >= i * NP) & (dst < (i + 1) * NP)
        s_i = src[m]
        ld_i = dst[m] - i * NP
        order = np.argsort(ld_i, kind="stable")
        s_i, ld_i = s_i[order], ld_i[order]
        rows = row_of[s_i]
        a = rows < HALF
        core_ld[i][0], core_idx[i][0] = ld_i[a], rows[a]
        core_ld[i][1], core_idx[i][1] = ld_i[~a], rows[~a] - HALF

    # re-align all cores' streams at every RESYNC_G windows: within a group,
    # pad each core's segment to the max core's tile count, so tile t sits in
    # the same window neighborhood on every core (cuts union-pair straddle).
    G = CFG.get("RESYNC_G", 16)
    n_groups = (WIN + G - 1) // G
    for s in range(2):
        seg_tiles = np.zeros(n_groups, dtype=np.int64)
        for g in range(n_groups):
            lo, hi = g * G * 128, min((g + 1) * G, WIN) * 128
            for i in range(C):
                cnt = int(((core_ld[i][s] >= lo) & (core_ld[i][s] < hi)).sum())
                seg_tiles[g] = max(seg_tiles[g], (cnt + 127) // 128)
        for i in range(C):
            lds, ixs = [], []
            for g in range(n_groups):
                lo, hi = g * G * 128, min((g + 1) * G, WIN) * 128
                m = (core_ld[i][s] >= lo) & (core_ld[i][s] < hi)
                ld_g, ix_g = core_ld[i][s][m], core_idx[i][s][m]
                pad = int(seg_tiles[g]) * 128 - len(ld_g)
                lds.append(np.concatenate([ld_g, np.full(pad, SENT, np.int64)]))
                ixs.append(np.concatenate([ix_g, np.zeros(pad, np.int64)]))
            core_ld[i][s] = np.concatenate(lds)
            core_idx[i][s] = np.concatenate(ixs)
    T = [len(core_ld[0][0]) // 128, len(core_ld[0][1]) // 128]
    pr.T = T

    for i in range(C):
        for s in range(2):
            assert len(core_ld[i][s]) == T[s] * 128

    # union pair structure (w, stream, tile) across cores
    pair_set = set()
    for i in range(C):
        for s in range(2):
            L = core_ld[i][s].reshape(T[s], 128)
            for t in range(T[s]):
                real = L[t][L[t] != SENT]
                if len(real) == 0:
                    continue
                for w in range(int(real.min()) // 128, int(real.max()) // 128 + 1):
                    pair_set.add((w, s, t))
    for w in range(WIN):  # every window needs >=1 pair so psum gets reset
        if not any(p[0] == w for p in pair_set):
            pair_set.add((w, 0, 0))
    pairs = sorted(pair_set)
    pr.pairs = pairs
    pr.n_pairs = len(pairs)
    segs = [[] for _ in range(WIN)]
    for k, (w, s, t) in enumerate(pairs):
        segs[w].append(k)
    pr.segs = segs

    # per-core one-hot tiles [128, n_pairs, 128] fp8(0/1)
    pr.onehot = []
    pr.idx_wrapped = []
    pr.deg_staged = []
    oh_sync = CFG.get("OH_SYNC", 0)
    BF16_ONE = 0x3F80  # bfloat16 bit pattern of 1.0
    one_bits = BF16_ONE if oh_sync else FP8_ONE
    oh_dt = np.uint16 if oh_sync else np.uint8
    oh_view = ml_dtypes.bfloat16 if oh_sync else ml_dtypes.float8_e4m3fn
    for i in range(C):
        oh = np.zeros((128, pr.n_pairs, 128), dtype=oh_dt)
        for k, (w, s, t) in enumerate(pairs):
            ld_t = core_ld[i][s][t * 128 : (t + 1) * 128]
            slot = ld_t - 128 * w
            valid = (slot >= 0) & (slot < 128)
            rr = np.nonzero(valid)[0]
            oh[rr, k, slot[rr]] = one_bits
        pr.onehot.append(oh.view(oh_view))

        blocks = []
        for s in range(2):
            ix = core_idx[i][s].astype(np.int16)
            assert (core_idx[i][s] < 32768).all() and (core_idx[i][s] >= 0).all()
            w16 = ix.reshape(-1, 16).T  # [16, T*8]
            blocks.append(np.tile(w16, (8, 1)))  # replicate to 128 partitions
        pr.idx_wrapped.append(
            np.ascontiguousarray(np.concatenate(blocks, axis=1))
        )

        dshard = deg[i * NP : (i + 1) * NP]
        pr.deg_staged.append(
            np.ascontiguousarray(dshard.reshape(WIN, 128).T.astype(np.float32))
        )

    return pr


# ------------------------------------------------------------------ bass ----


def _build(pr):
    import concourse.bass as bass
    import concourse.bacc as bacc
    import concourse.mybir as mybir
    import concourse.tile as tile
    from concourse._compat import get_trn_type

    dt = mybir.dt
    Alu = mybir.AluOpType
    F32, BF16, FP8, I16 = dt.float32, dt.bfloat16, dt.float8e4, dt.int16

    IN_C, OUT_C = CFG["IN_C"], CFG["OUT_C"]
    OC_PAD, CH, CHP = CFG["OC_PAD"], CFG["CH"], CFG["CHP"]
    N, C, NP, WIN, HALF = pr.N, pr.C, pr.NP, pr.WIN, pr.HALF
    T = pr.T

    nc = bacc.Bacc(
        get_trn_type() or "TRN2",
        target_bir_lowering=False,
        debug=False,
        num_devices=C,
        num_swdge_queues=CFG.get("NQ", 1),
    )

    xt_d = nc.dram_tensor("xt", [IN_C, NP], F32, kind="ExternalInput")
    wt_d = nc.dram_tensor("wt", [IN_C, OUT_C], F32, kind="ExternalInput")
    b_d = nc.dram_tensor("bias", [128, OUT_C], F32, kind="ExternalInput")
    deg_d = nc.dram_tensor("deg", [128, WIN], F32, kind="ExternalInput")
    idx_d = nc.dram_tensor(
        "idx", [128, (T[0] + T[1]) * 8], I16, kind="ExternalInput"
    )
    OH_DT = BF16 if CFG.get("OH_SYNC", 0) else FP8
    oh_d = nc.dram_tensor("oh", [128, pr.n_pairs, 128], OH_DT, kind="ExternalInput")
    out_d = nc.dram_tensor("out", [NP, OUT_C], F32, kind="ExternalOutput")

    rg = [list(range(C))]

    with tile.TileContext(nc) as tc:
        with (
            tc.tile_pool(name="const", bufs=1) as const,
            tc.tile_pool(name="dram", bufs=1, space="DRAM") as dram,
            tc.tile_pool(name="psum_y", bufs=2, space="PSUM") as psum_y,
            tc.tile_pool(name="psum_w", bufs=CFG["PSUM_BUFS"], space="PSUM") as psum_w,
            tc.tile_pool(name="msgA", bufs=CFG["MSG_BUFS"]) as msgA_pool,
            tc.tile_pool(name="msgB", bufs=CFG["MSG_BUFS"]) as msgB_pool,
            tc.tile_pool(name="ohp", bufs=CFG["OH_BUFS"]) as oh_pool,
            tc.tile_pool(name="tmp", bufs=4) as tmp_pool,
        ):
            cc1_in = dram.tile([NP, OC_PAD], BF16)
            cc1_out = dram.tile([N, OC_PAD], BF16, addr_space="Shared")
            cc2_in = dram.tile([NP, OC_PAD], BF16)
            cc2_out = dram.tile([N, OC_PAD], BF16, addr_space="Shared")

            idx_sb = const.tile([128, (T[0] + T[1]) * 8], I16)
            nc.sync.dma_start(idx_sb[:], idx_d[:])
            wt_sb = const.tile([IN_C, OUT_C], F32)
            nc.sync.dma_start(wt_sb[:], wt_d[:])
            b_sb = const.tile([128, OUT_C], F32)
            nc.sync.dma_start(b_sb[:], b_d[:])
            deg_sb = const.tile([128, WIN], F32)
            nc.sync.dma_start(deg_sb[:], deg_d[:])
            xt_sb = const.tile([IN_C, NP], F32)
            nc.sync.dma_start(xt_sb[:], xt_d[:])

            deginv = const.tile([128, WIN], F32)
            nc.vector.reciprocal(deginv[:], deg_sb[:])
            dinv = const.tile([128, WIN], F32)
            nc.scalar.activation(
                dinv[:], deginv[:], mybir.ActivationFunctionType.Sqrt
            )

            z0f = const.tile([128, WIN, OUT_C], F32)
            z1f = const.tile([128, WIN, OUT_C], F32)
            outst = const.tile([128, WIN, OUT_C], F32)
            zpad1 = const.tile([128, WIN, OC_PAD], BF16)
            zpad2 = const.tile([128, WIN, OC_PAD], BF16)
            nc.vector.memset(zpad1[:], 0.0)
            nc.vector.memset(zpad2[:], 0.0)

            # ---- projection: z0 = dinv * (x @ W^T), staged [p, w, ch] ----
            for r in range(WIN):
                py = psum_y.tile([128, OUT_C], F32)
                nc.tensor.matmul(
                    py[:],
                    xt_sb[:, r * 128 : (r + 1) * 128],
                    wt_sb[:],
                    start=True,
                    stop=True,
                )
                if CFG["ACT_EVAC"]:
                    nc.scalar.mul(z0f[:, r, :], py[:], dinv[:, r : r + 1])
                    nc.scalar.copy(zpad1[:, r, 0:OUT_C], z0f[:, r, :])
                else:
                    nc.vector.tensor_scalar(
                        z0f[:, r, :], py[:], dinv[:, r : r + 1], None, Alu.mult
                    )
                    nc.vector.tensor_copy(zpad1[:, r, 0:OUT_C], z0f[:, r, :])
            STAGE = CFG["STAGE"]
            if STAGE >= 2:
                nc.sync.dma_start(cc1_in[:], zpad1[:])
                nc.gpsimd.collective_compute(
                    "AllGather",
                    Alu.bypass,
                    replica_groups=rg,
                    ins=[cc1_in.opt()],
                    outs=[cc1_out.opt()],
                )

            calls = [(T[0] + CH - 1) // CH, (T[1] + CH - 1) // CH]
            n_oh_chunks = (pr.n_pairs + CHP - 1) // CHP
            colbase = [0, T[0] * 8]

            NQ = CFG.get("NQ", 1)
            gq_counter = [0]

            def run_hop(cc_out, evac, do_mm=True):
                tabs = [cc_out[0:HALF, :], cc_out[HALF : 2 * HALF, :]]
                pools = [msgA_pool, msgB_pool]
                msg_tiles = [{}, {}]
                oh_tiles = {}
                next_call = [0, 0]
                next_oh = [0]

                def emit_gather(s):
                    c = next_call[s]
                    ntiles = min(CH, T[s] - c * CH)
                    ni = ntiles * 128
                    t = pools[s].tile([128, CH, OC_PAD], BF16, tag=f"msg{s}")
                    sl = slice(colbase[s] + c * CH * 8, colbase[s] + c * CH * 8 + ntiles * 8)
                    nc.gpsimd.dma_gather(
                        t[:, 0:ntiles, :],
                        tabs[s],
                        idx_sb[:, sl],
                        ni,
                        ni,
                        OC_PAD,
                        single_packet=(ni <= 1024),
                        queue_num=gq_counter[0] % NQ,
                    )
                    gq_counter[0] += 1
                    msg_tiles[s][c] = t
                    next_call[s] = c + 1

                def emit_oh():
                    k = next_oh[0]
                    npair = min(CHP, pr.n_pairs - k * CHP)
                    t = oh_pool.tile([128, CHP, 128], BF16, tag="oh")
                    oh_eng = nc.sync if CFG.get("OH_SYNC", 0) else nc.gpsimd
                    oh_eng.dma_start(
                        out=t[:, 0:npair, :],
                        in_=oh_d[:, k * CHP : k * CHP + npair, :],
                    )
                    oh_tiles[k] = t
                    next_oh[0] = k + 1

                for w in range(WIN):
                    seg = pr.segs[w]
                    # make sure resources (plus one chunk of prefetch) exist
                    for pk in seg:
                        _, s, t = pr.pairs[pk]
                        while next_call[s] <= min(t // CH + CFG.get("PF", 1), calls[s] - 1):
                            emit_gather(s)
                        while next_oh[0] <= min(pk // CHP + 1, n_oh_chunks - 1):
                            emit_oh()
                    if not do_mm:
                        continue
                    pw = psum_w.tile([128, OUT_C], F32)
                    for j, pk in enumerate(seg):
                        _, s, t = pr.pairs[pk]
                        oh_ap = oh_tiles[pk // CHP][:, pk % CHP, :]
                        msg_ap = msg_tiles[s][t // CH][:, t % CH, 0:OUT_C]
                        nc.tensor.matmul(
                            pw[:],
                            oh_ap,
                            msg_ap,
                            start=(j == 0),
                            stop=(j == len(seg) - 1),
                        )
                    evac(w, pw)

            # ---- hop 1:  z1 = (psum + z0) / deg ----
            def evac1(w, pw):
                tmp = tmp_pool.tile([128, OUT_C], F32, tag="tmp")
                nc.vector.tensor_add(tmp[:], pw[:], z0f[:, w, :])
                if CFG["ACT_EVAC"]:
                    nc.scalar.mul(z1f[:, w, :], tmp[:], deginv[:, w : w + 1])
                    nc.scalar.mul(
                        zpad2[:, w, 0:OUT_C], tmp[:], deginv[:, w : w + 1]
                    )
                else:
                    nc.vector.tensor_scalar(
                        z1f[:, w, :], tmp[:], deginv[:, w : w + 1], None, Alu.mult
                    )
                    nc.vector.tensor_copy(zpad2[:, w, 0:OUT_C], z1f[:, w, :])

            if STAGE >= 3:
                run_hop(cc1_out, evac1, do_mm=STAGE >= 4)
            if STAGE >= 5:
                nc.sync.dma_start(cc2_in[:], zpad2[:])
                nc.gpsimd.collective_compute(
                    "AllGather",
                    Alu.bypass,
                    replica_groups=rg,
                    ins=[cc2_in.opt()],
                    outs=[cc2_out.opt()],
                )

            # ---- hop 2:  out = dinv * (psum + z1) + b ----
            def evac2(w, pw):
                tmp = tmp_pool.tile([128, OUT_C], F32, tag="tmp")
                tmp2 = tmp_pool.tile([128, OUT_C], F32, tag="tmp2")
                nc.vector.tensor_add(tmp[:], pw[:], z1f[:, w, :])
                if CFG["ACT_EVAC"]:
                    nc.scalar.mul(tmp2[:], tmp[:], dinv[:, w : w + 1])
                else:
                    nc.vector.tensor_scalar(
                        tmp2[:], tmp[:], dinv[:, w : w + 1], None, Alu.mult
                    )
                nc.vector.tensor_add(outst[:, w, :], tmp2[:], b_sb[:])

            if STAGE >= 6:
                run_hop(cc2_out, evac2)
            src_final = {1: z0f, 2: z0f, 3: z0f, 4: z1f, 5: z1f, 6: outst}[STAGE]
            nc.sync.dma_start(out_d[:], src_final[:])

    nc.compile()
    return nc


def _make_in_maps(pr, x, W, b):
    C, NP, WIN = pr.C, pr.NP, pr.WIN
    x = np.asarray(x, dtype=np.float32)
    W = np.asarray(W, dtype=np.float32)
    b = np.asarray(b, dtype=np.float32)
    wt = np.ascontiguousarray(W.T)
    b_rep = np.ascontiguousarray(np.broadcast_to(b, (128, len(b))))
    in_maps = []
    for i in range(C):
        xt = np.ascontiguousarray(x[i * NP : (i + 1) * NP].T)
        in_maps.append(
            dict(
                xt=xt,
                wt=wt,
                bias=b_rep,
                deg=pr.deg_staged[i],
                idx=pr.idx_wrapped[i],
                oh=pr.onehot[i],
            )
        )
    return in_maps


def _unpermute(o, pr):
    # device rows are p*WIN+w; node order is w*128+p
    return (
        o.reshape(128, pr.WIN, o.shape[-1])
        .transpose(1, 0, 2)
        .reshape(pr.NP, o.shape[-1])
    )


_CACHE = {}


def kernel(x, edge_index, W, b):
    pr = _preprocess(edge_index)
    nc = _build(pr)
    in_maps = _make_in_maps(pr, x, W, b)

    from concourse import bass_utils

    res = bass_utils.run_bass_kernel_spmd(
        nc, in_maps, core_ids=list(range(pr.C))
    )
    shards = [_unpermute(res.results[i]["out"], pr) for i in range(pr.C)]
    return np.ascontiguousarray(np.concatenate(shards, axis=0))

